# revision 13
# baseline (speedup 1.0000x reference)
"""CNN-LSTM Trainium2 kernel: 8-way tensor-parallel over the 4H gate dim.

v2: split-exchange schedule.
- Host folds the hidden projection into the gate weights (M00 = W_hh0 @ W_hr0,
  M10 = W_ih1 @ W_hr0, M11 = W_hh1 @ W_hr1) so the recurrence runs entirely on
  the sharded s = sigmoid(o)*tanh(c) vectors (H=1024, 128 per core).
- Gate columns per core ordered [i | f | o | g] so one Sigmoid op covers
  i,f,o (384 cols) and one Tanh covers g.
- Two 16KB broadcasts per superstep: A carries s0^T(n) (triggered mid-superstep
  right after the L0 cell update), B carries s1^T(n) (triggered early superstep
  n+1).  The L0 recurrence cycle (arrival -> 8 MMs -> ACT/DVE -> transpose ->
  copy -> trigger -> flight) no longer waits on the L1 chain.
- PE program per superstep: x-part, g1 s0-part (old exchange), transpose-B,
  g1 rest, D-g0 (on arrival-A), g1 s1-part, bias, transpose-A.  Fill work
  precedes the arrival wait to keep HAM warm.
- Epilogue: h1 = P1 @ s1 + softmax, sharded over T (16 steps/core).
"""
import sys
import os
import numpy as np

sys.path.insert(0, "/opt/trn_rl_repo")

import concourse.bass as bass  # noqa: E402
import concourse.bacc as bacc  # noqa: E402
import concourse.mybir as mybir  # noqa: E402
from concourse.bass_utils import run_bass_kernel_spmd  # noqa: E402
import ml_dtypes  # noqa: E402

BF = mybir.dt.bfloat16
F32 = mybir.dt.float32
AF = mybir.ActivationFunctionType

B, T, E, H, V = 64, 128, 512, 1024, 10000
NCORES = 8
TRACE = False
LAST_EXEC_NS = None
_CACHE = {}


def _install_trace_hook():
    import types, contextlib, ctypes

    if "antenv.axon_hooks" in sys.modules:
        return
    mod = types.ModuleType("antenv.axon_hooks")
    mod._hook = None
    mod.set_axon_ntff_profile_hook = lambda h: setattr(mod, "_hook", h)
    mod.get_axon_ntff_profile_hook = lambda: mod._hook
    sys.modules["antenv.axon_hooks"] = mod
    import antenv

    antenv.axon_hooks = mod
    so_path = "/opt/axon/libaxon_pjrt.so"
    try:
        lib = ctypes.CDLL(so_path)
    except OSError:
        return
    if not hasattr(lib, "axon_start_nrt_profile"):
        return
    lib.axon_start_nrt_profile.argtypes = [ctypes.POINTER(ctypes.c_int64), ctypes.c_size_t]
    lib.axon_start_nrt_profile.restype = ctypes.c_int64
    lib.axon_stop_nrt_profile.argtypes = [ctypes.c_char_p]
    lib.axon_stop_nrt_profile.restype = ctypes.c_int64

    @contextlib.contextmanager
    def _hook(output_dir, device_ids):
        import jax

        jax.devices()
        if device_ids:
            ids = (ctypes.c_int64 * len(device_ids))(*device_ids)
            rc = lib.axon_start_nrt_profile(ids, len(device_ids))
        else:
            rc = lib.axon_start_nrt_profile(None, 0)
        if rc != 0:
            raise RuntimeError(f"axon_start_nrt_profile rc={rc}")
        try:
            yield
        finally:
            n = lib.axon_stop_nrt_profile(str(output_dir).encode())
            print(f"profile: {n} file(s) -> {output_dir}", file=sys.stderr)

    mod.set_axon_ntff_profile_hook(_hook)


def build(t_steps=T, dump=False):
    NS = t_steps + 3  # supersteps 0 .. t_steps+2
    TSH = t_steps // NCORES  # epilogue steps per core

    nc = bacc.Bacc("TRN2", target_bir_lowering=False, debug=False, num_devices=8)

    # ---- I/O ----
    w0d = nc.dram_tensor("w0", [13, 128, 512], BF, kind="ExternalInput")
    w1d = nc.dram_tensor("w1", [17, 128, 512], BF, kind="ExternalInput")
    p1d = nc.dram_tensor("p1w", [8, 128, 512], BF, kind="ExternalInput")
    xtd = nc.dram_tensor("xT", [512, t_steps * 64], BF, kind="ExternalInput")
    onesd = nc.dram_tensor("ones", [128, 64], BF, kind="ExternalInput")
    idend = nc.dram_tensor("iden", [64, 64], BF, kind="ExternalInput")
    rankd = nc.dram_tensor("rank", [1, 1], mybir.dt.int32, kind="ExternalInput")
    rank16d = nc.dram_tensor("rank16", [1, 1], mybir.dt.int32, kind="ExternalInput")
    yd = nc.dram_tensor("y", [64, TSH, 512], F32, kind="ExternalOutput")
    s1store = nc.dram_tensor(
        "s1store", [t_steps, 128 * 512], BF,
        kind="ExternalOutput" if dump else "Internal",
    )

    # ---- SBUF ----
    W0 = nc.alloc_sbuf_tensor("W0", [128, 13 * 512], BF)
    W1 = nc.alloc_sbuf_tensor("W1", [128, 17 * 512], BF)
    P1S = nc.alloc_sbuf_tensor("P1S", [128, 8 * 512], BF)
    Gb = [nc.alloc_sbuf_tensor(f"G{q}", [128, 1024], BF) for q in range(3)]
    SSA = [nc.alloc_sbuf_tensor(f"SSA{p}", [128, 64], BF) for p in range(2)]
    SSB = [nc.alloc_sbuf_tensor(f"SSB{p}", [128, 64], BF) for p in range(2)]
    XT = nc.alloc_sbuf_tensor("XT", [128, 2 * 256], BF)
    ONES = nc.alloc_sbuf_tensor("ONES", [128, 64], BF)
    IDN = nc.alloc_sbuf_tensor("IDN", [64, 64], BF)
    actb = nc.alloc_sbuf_tensor("actb", [64, 1024], F32)  # [i f o g] x 2 layers
    cbuf = nc.alloc_sbuf_tensor("cbuf", [64, 256], F32)  # c0 | c1
    thc = nc.alloc_sbuf_tensor("thc", [64, 256], F32)  # tanh(c0) | tanh(c1)
    sS = [nc.alloc_sbuf_tensor(f"sS{p}", [64, 256], BF) for p in range(2)]  # s0|s1
    es1 = [nc.alloc_sbuf_tensor(f"es1_{p}", [128, 512], BF) for p in range(2)]
    emx = nc.alloc_sbuf_tensor("emx", [64, 8], F32)  # max, negmax, sum, rsum slots
    ebuf = nc.alloc_sbuf_tensor("ebuf", [64, 512], F32)

    # ---- PSUM (8 banks total) ----
    ps_g0 = [nc.alloc_psum_tensor(f"psg0_{p}", [64, 512], F32) for p in range(2)]
    ps_g1 = [nc.alloc_psum_tensor(f"psg1_{p}", [64, 512], F32) for p in range(2)]
    ps_t = [nc.alloc_psum_tensor(f"pst_{p}", [128, 128], BF) for p in range(2)]
    ps_e = [nc.alloc_psum_tensor(f"pse_{p}", [64, 512], F32) for p in range(2)]

    # ---- semaphores ----
    rsA = [nc.alloc_semaphore(f"rsA{q}") for q in range(3)]
    rsB = [nc.alloc_semaphore(f"rsB{q}") for q in range(3)]
    prep = nc.alloc_semaphore("prep")
    lsemA = nc.alloc_semaphore("lsemA")
    lsemB = nc.alloc_semaphore("lsemB")
    pe = nc.alloc_semaphore("pe")
    acts = nc.alloc_semaphore("acts")
    dve = nc.alloc_semaphore("dve")
    xdma = nc.alloc_semaphore("xdma")
    sdma = nc.alloc_semaphore("sdma")
    edma = nc.alloc_semaphore("edma")
    idma = nc.alloc_semaphore("idma")
    init = nc.alloc_semaphore("init")
    ydma = nc.alloc_semaphore("ydma")

    rdests = [(0, k) for k in range(8)]

    # ---- schedule predicates ----
    def A_ex(m):
        return 0 <= m <= t_steps - 1

    def B_ex(m):
        return 2 <= m <= t_steps + 1

    def L0(n):
        return n <= t_steps - 1

    def D0(n):
        return 1 <= n <= t_steps - 1

    def L1(n):
        return 2 <= n <= t_steps + 1

    def B2(n):
        return 3 <= n <= t_steps + 1

    def TA(n):
        return A_ex(n)

    def TBp(n):  # transpose/copy/trigger slot for B(n-1)
        return B_ex(n - 1)

    def rthA(m):
        return 16 * (m // 3 + 1)

    def rthB(m):
        return 16 * ((m - 2) // 3 + 1)

    # ---- analytic milestone tables ----
    # PE program order per n: x(n)+bias, b1a(n), TB(n-1), b1b(n), D-g0(n),
    #                         b2a(n), TA(n), b2b(n)+bias
    pe_tb, pe_g0, pe_g1, pe_ta = {}, {}, {}, {}
    a_sig0, a_tg0, a_tc0, a_sig1, a_tg1, a_tc1 = {}, {}, {}, {}, {}, {}
    d_cpB, d_c0, d_s0, d_cpA, d_c1, d_s1 = {}, {}, {}, {}, {}, {}
    prep_A, prep_B = {}, {}
    c_pe = c_a = c_d = 0
    for n in range(NS):
        if L0(n) and not D0(n):  # n == 0: bias stop carries the inc
            c_pe += 1
            pe_g0[0] = c_pe
        if TBp(n):
            c_pe += 1
        pe_tb[n] = c_pe
        if D0(n):
            c_pe += 1
            pe_g0[n] = c_pe
        if TA(n):
            c_pe += 1
        pe_ta[n] = c_pe
        if L1(n):
            c_pe += 1
        pe_g1[n] = c_pe

        if L0(n):
            c_a += 1
            a_sig0[n] = c_a
            c_a += 1
            a_tg0[n] = c_a
            c_a += 1
            a_tc0[n] = c_a
        if L1(n):
            c_a += 1
            a_sig1[n] = c_a
            c_a += 1
            a_tg1[n] = c_a
            c_a += 1
            a_tc1[n] = c_a

        # DVE order per n: cpB | m1,m2,add | s0 | cpA | c1-ops | s1
        if TBp(n):
            c_d += 1
        d_cpB[n] = c_d
        if L0(n):
            c_d += 1
            d_c0[n] = c_d
            c_d += 1
            d_s0[n] = c_d
        if TA(n):
            c_d += 1
        d_cpA[n] = c_d
        if L1(n):
            c_d += 1
            d_c1[n] = c_d
            c_d += 1
            d_s1[n] = c_d
    PE_END, ACT_END, DVE_END = c_pe, c_a, c_d

    # prep FIFO counter (A(0) prepped in prologue)
    c_p = 1
    prep_A[0] = 1
    for n in range(NS):
        if B_ex(n):
            c_p += 1
            prep_B[n] = c_p
        if A_ex(n + 1):
            c_p += 1
            prep_A[n + 1] = c_p

    # store count through B(m) (stores happen at superstep m+1)
    def st_cnt(m):  # number of s1store writes for B(2..m)
        return max(0, min(m, t_steps + 1) - 1) if m >= 2 else 0

    n_stores = st_cnt(t_steps + 1)  # == t_steps

    with nc.Block() as block:

        # ================= GPSIMD =================
        @block.gpsimd
        def _(g):
            with g.register("rank") as rank, g.register("urow") as urow, \
                    g.register("r16") as r16:
                g.load(rank, rankd.ap())
                g.load(r16, rank16d.ap())
                g.dma_start(
                    out=W0.rearrange("p (k c) -> p k c", k=13),
                    in_=w0d.rearrange("k p c -> p k c"),
                ).then_inc(idma, 16)
                g.dma_start(
                    out=W1.rearrange("p (k c) -> p k c", k=17),
                    in_=w1d.rearrange("k p c -> p k c"),
                ).then_inc(idma, 16)
                g.dma_start(
                    out=P1S.rearrange("p (k c) -> p k c", k=8),
                    in_=p1d.rearrange("k p c -> p k c"),
                ).then_inc(idma, 16)
                g.dma_start(out=ONES[:, :], in_=onesd[:, :]).then_inc(idma, 16)
                g.dma_start(out=IDN[:, :], in_=idend[:, :]).then_inc(idma, 16)
                g.wait_ge(idma, 80)
                g.memset(cbuf[:, :], 0.0)
                g.memset(emx[:, :], 0.0).then_inc(init, 1)
                g.bir_kernel_barrier_wait([list(range(8))])
                # prologue: prep A(0)
                for r in range(8):
                    with g.If_eq(rank, r):
                        g.remote_dma_broadcast(
                            out_ap=Gb[0][:, r * 128:r * 128 + 64],
                            in_ap=SSA[0][:, :],
                            remote_sem=rsA[0],
                            local_sem=lsemA,
                            rdests=rdests,
                        ).then_inc(prep, 1)
                for n in range(NS):
                    # --- triggers first: B(n-1) then A(n) ---
                    if TBp(n):
                        g.wait_ge(dve, d_cpB[n])
                        g.wait_ge(prep, prep_B[n - 1])
                        if n >= 6:
                            g.wait_ge(sdma, 16 * st_cnt(n - 4))
                        g.trigger_dma(count=1)
                    if TA(n):
                        g.wait_ge(dve, d_cpA[n])
                        g.wait_ge(prep, prep_A[n])
                        g.trigger_dma(count=1)
                    # --- preps: B(n) then A(n+1) (FIFO matches trigger order) ---
                    if B_ex(n):
                        p3, p2 = n % 3, n % 2
                        for r in range(8):
                            with g.If_eq(rank, r):
                                g.remote_dma_broadcast(
                                    out_ap=Gb[p3][:, r * 128 + 64:(r + 1) * 128],
                                    in_ap=SSB[p2][:, :],
                                    remote_sem=rsB[(n - 2) % 3],
                                    local_sem=lsemB,
                                    rdests=rdests,
                                ).then_inc(prep, 1)
                    if A_ex(n + 1):
                        p3, p2 = (n + 1) % 3, (n + 1) % 2
                        for r in range(8):
                            with g.If_eq(rank, r):
                                g.remote_dma_broadcast(
                                    out_ap=Gb[p3][:, r * 128:r * 128 + 64],
                                    in_ap=SSA[p2][:, :],
                                    remote_sem=rsA[p3],
                                    local_sem=lsemA,
                                    rdests=rdests,
                                ).then_inc(prep, 1)
                # ---- epilogue input DMAs ----
                g.wait_ge(sdma, 16 * n_stores)  # all s1 stores landed
                for j in range(TSH):
                    g.reg_add(urow, r16, j)
                    if j >= 2:
                        g.wait_ge(pe, PE_END + j - 1)  # es1[j%2] WAR
                    g.dma_start(
                        out=es1[j % 2][:, :],
                        in_=s1store[bass.ds(g.snap(urow), 1), :].rearrange(
                            "a (p c) -> (a p) c", p=128
                        ),
                    ).then_inc(edma, 16)

        # ================= SYNC (HWDGE staging/stores) =================
        @block.sync
        def _(sy):
            sy.wait_ge(init, 1)
            sy.dma_start(
                out=XT[:, 0:256].rearrange("p (a c) -> p a c", a=4),
                in_=xtd.rearrange("(a p) t -> p a t", p=128)[:, :, 0:64],
            ).then_inc(xdma, 16)
            for n in range(NS):
                m = n + 1  # load XT[m] during iteration n (used in iter m)
                if 1 <= m <= t_steps - 1:
                    if n >= 1:
                        sy.wait_ge(pe, pe_g0[n - 1])  # implies x(n-1) done
                    sy.dma_start(
                        out=XT[:, (m % 2) * 256:(m % 2 + 1) * 256]
                        .rearrange("p (a c) -> p a c", a=4),
                        in_=xtd.rearrange("(a p) t -> p a t", p=128)[
                            :, :, m * 64:(m + 1) * 64
                        ],
                    ).then_inc(xdma, 16)
                if B_ex(n - 1):  # store s1 of B(n-1) at superstep n
                    m = n - 1
                    sy.wait_ge(rsB[(m - 2) % 3], rthB(m))
                    sy.dma_start(
                        out=s1store[m - 2, :].rearrange(
                            "(p k c) -> p k c", p=128, k=8
                        ),
                        in_=Gb[m % 3].rearrange("p (k c) -> p k c", k=8)[
                            :, :, 64:128
                        ],
                    ).then_inc(sdma, 16)
            # epilogue output DMAs
            for j in range(TSH):
                sy.wait_ge(dve, DVE_END + j * 4 + 4)
                sy.dma_start(out=yd[:, j, :], in_=ebuf[:, :]).then_inc(ydma, 16)

        # ================= TENSOR (PE) =================
        def emit_x(te, m):
            """x-part + bias for L0 step m into ps_g0[m%2]."""
            q2 = m % 2
            te.wait_ge(xdma, 16 * (m + 1))
            if m >= 2:
                te.wait_ge(acts, a_tg0[m - 2])  # ps_g0[q2] WAR
            for k in range(4):
                te.matmul(
                    ps_g0[q2][:, :],
                    XT[:, q2 * 256 + k * 64:q2 * 256 + (k + 1) * 64],
                    W0[:, k * 512:(k + 1) * 512],
                    start=(k == 0), stop=False,
                )
            mm = te.matmul(
                ps_g0[q2][:, :], ONES[:, :], W0[:, 12 * 512:13 * 512],
                start=False, stop=(not D0(m)),
            )
            if not D0(m):
                mm.then_inc(pe, 1)

        @block.tensor
        def _(te):
            te.wait_ge(init, 1)
            for n in range(NS):
                p2, p3 = n % 2, n % 3
                # --- 1: x-part + bias for step n ---
                if L0(n):
                    emit_x(te, n)
                # --- 2: g1 s0-part first chunk (exchange n-2) ---
                if L1(n):
                    te.wait_ge(rsA[(n - 2) % 3], rthA(n - 2))
                    if n >= 4 and L1(n - 2):
                        te.wait_ge(acts, a_tg1[n - 2])  # ps_g1[p2] WAR
                    for k in range(3):
                        te.matmul(
                            ps_g1[p2][:, :],
                            Gb[(n - 2) % 3][:, k * 128:k * 128 + 64],
                            W1[:, k * 512:(k + 1) * 512],
                            start=(k == 0), stop=False,
                        )
                # --- 3: transpose-B of s1(n-1) ---
                if TBp(n):
                    te.wait_ge(dve, d_s1[n - 1])
                    te.transpose(
                        ps_t[(n - 1) % 2][:, 64:128],
                        sS[(n - 1) % 2][:, 128:256], IDN[:, :],
                    ).then_inc(pe, 1)
                # --- 4: g1 s0-part rest ---
                if L1(n):
                    for k in range(3, 8):
                        te.matmul(
                            ps_g1[p2][:, :],
                            Gb[(n - 2) % 3][:, k * 128:k * 128 + 64],
                            W1[:, k * 512:(k + 1) * 512],
                            start=False, stop=False,
                        )
                # --- 5: D-g0 s-part (exchange n-1) ---
                if D0(n):
                    te.wait_ge(rsA[(n - 1) % 3], rthA(n - 1))
                    for k in range(8):
                        mm = te.matmul(
                            ps_g0[p2][:, :],
                            Gb[(n - 1) % 3][:, k * 128:k * 128 + 64],
                            W0[:, (4 + k) * 512:(5 + k) * 512],
                            start=False, stop=(k == 7),
                        )
                        if k == 7:
                            mm.then_inc(pe, 1)
                # --- 6: g1 s1-part first 6 (exchange B n-1) ---
                if L1(n):
                    if B2(n):
                        te.wait_ge(rsB[(n - 3) % 3], rthB(n - 1))
                        for k in range(6):
                            te.matmul(
                                ps_g1[p2][:, :],
                                Gb[(n - 1) % 3][:, k * 128 + 64:(k + 1) * 128],
                                W1[:, (8 + k) * 512:(9 + k) * 512],
                                start=False, stop=False,
                            )
                # --- 7: transpose-A of s0(n) ---
                if TA(n):
                    te.wait_ge(dve, d_s0[n])
                    te.transpose(
                        ps_t[p2][:, 0:64],
                        sS[p2][:, 0:128], IDN[:, :],
                    ).then_inc(pe, 1)
                # --- 8: g1 s1-part rest + bias ---
                if L1(n):
                    if B2(n):
                        for k in range(6, 8):
                            te.matmul(
                                ps_g1[p2][:, :],
                                Gb[(n - 1) % 3][:, k * 128 + 64:(k + 1) * 128],
                                W1[:, (8 + k) * 512:(9 + k) * 512],
                                start=False, stop=False,
                            )
                    te.matmul(
                        ps_g1[p2][:, :], ONES[:, :], W1[:, 16 * 512:17 * 512],
                        start=False, stop=True,
                    ).then_inc(pe, 1)
            # ---- epilogue GEMMs ----
            for j in range(TSH):
                te.wait_ge(edma, 16 * (j + 1))
                if j >= 2:
                    te.wait_ge(acts, ACT_END + j * 2 - 2)  # ps_e WAR
                mm_e = None
                for k in range(8):
                    mm_e = te.matmul(
                        ps_e[j % 2][:, :],
                        es1[j % 2][:, k * 64:(k + 1) * 64],
                        P1S[:, k * 512:(k + 1) * 512],
                        start=(k == 0), stop=(k == 7),
                    )
                mm_e.then_inc(pe, 1)

        # ================= SCALAR (ACT) =================
        @block.scalar
        def _(sc):
            for n in range(NS):
                p2 = n % 2
                if L0(n):
                    sc.wait_ge(pe, pe_g0[n])
                    sc.activation(
                        actb[:, 0:384], ps_g0[p2][:, 0:384], AF.Sigmoid
                    ).then_inc(acts, 1)
                    sc.activation(
                        actb[:, 384:512], ps_g0[p2][:, 384:512], AF.Tanh
                    ).then_inc(acts, 1)
                    sc.wait_ge(dve, d_c0[n])
                    sc.activation(
                        thc[:, 0:128], cbuf[:, 0:128], AF.Tanh
                    ).then_inc(acts, 1)
                if L1(n):
                    sc.wait_ge(pe, pe_g1[n])
                    sc.activation(
                        actb[:, 512:896], ps_g1[p2][:, 0:384], AF.Sigmoid
                    ).then_inc(acts, 1)
                    sc.activation(
                        actb[:, 896:1024], ps_g1[p2][:, 384:512], AF.Tanh
                    ).then_inc(acts, 1)
                    sc.wait_ge(dve, d_c1[n])
                    sc.activation(
                        thc[:, 128:256], cbuf[:, 128:256], AF.Tanh
                    ).then_inc(acts, 1)
            # epilogue: negmax + exp
            for j in range(TSH):
                sc.wait_ge(dve, DVE_END + j * 4 + 1)
                if j >= 1:
                    sc.wait_ge(ydma, 16 * j)  # ebuf WAR vs output DMA
                sc.activation(
                    emx[:, 1:2], emx[:, 0:1], AF.Copy, scale=-1.0
                ).then_inc(acts, 1)
                sc.wait_ge(acts, ACT_END + j * 2 + 1)
                sc.activation(
                    ebuf[:, :], ps_e[j % 2][:, :], AF.Exp, bias=emx[:, 1:2]
                ).then_inc(acts, 1)

        # ================= VECTOR (DVE) =================
        @block.vector
        def _(ve):
            for n in range(NS):
                p2 = n % 2
                # copy-B of s1(n-1)
                if TBp(n):
                    ve.wait_ge(pe, pe_tb[n])
                    if n >= 5:
                        ve.wait_ge(lsemB, 16 * (n - 4))  # SSB[(n-1)%2] WAR
                    ve.tensor_copy(
                        SSB[(n - 1) % 2][:, :], ps_t[(n - 1) % 2][:, 64:128]
                    ).then_inc(dve, 1)
                # L0 cell update
                if L0(n):
                    ve.wait_ge(acts, a_sig0[n])
                    ve.tensor_mul(cbuf[:, 0:128], actb[:, 128:256], cbuf[:, 0:128])
                    ve.wait_ge(acts, a_tg0[n])
                    ve.tensor_mul(actb[:, 0:128], actb[:, 0:128], actb[:, 384:512])
                    ve.tensor_add(
                        cbuf[:, 0:128], cbuf[:, 0:128], actb[:, 0:128]
                    ).then_inc(dve, 1)
                    ve.wait_ge(acts, a_tc0[n])
                    ve.tensor_mul(
                        sS[p2][:, 0:128], actb[:, 256:384], thc[:, 0:128]
                    ).then_inc(dve, 1)
                # copy-A of s0(n)
                if TA(n):
                    ve.wait_ge(pe, pe_ta[n])
                    if n >= 2:
                        ve.wait_ge(lsemA, 16 * (n - 1))  # SSA[p2] WAR
                    ve.tensor_copy(
                        SSA[p2][:, :], ps_t[p2][:, 0:64]
                    ).then_inc(dve, 1)
                # L1 cell update
                if L1(n):
                    ve.wait_ge(acts, a_sig1[n])
                    ve.tensor_mul(cbuf[:, 128:256], actb[:, 640:768], cbuf[:, 128:256])
                    ve.wait_ge(acts, a_tg1[n])
                    ve.tensor_mul(actb[:, 512:640], actb[:, 512:640], actb[:, 896:1024])
                    ve.tensor_add(
                        cbuf[:, 128:256], cbuf[:, 128:256], actb[:, 512:640]
                    ).then_inc(dve, 1)
                    ve.wait_ge(acts, a_tc1[n])
                    ve.tensor_mul(
                        sS[p2][:, 128:256], actb[:, 768:896], thc[:, 128:256]
                    ).then_inc(dve, 1)
            # epilogue: max, sum, scale
            for j in range(TSH):
                ve.wait_ge(pe, PE_END + j + 1)
                if j >= 1:
                    ve.wait_ge(acts, ACT_END + j * 2 - 1)
                ve.tensor_reduce(
                    emx[:, 0:1], ps_e[j % 2][:, :],
                    mybir.AxisListType.X, mybir.AluOpType.max,
                ).then_inc(dve, 1)
                ve.wait_ge(acts, ACT_END + j * 2 + 2)
                ve.tensor_reduce(
                    emx[:, 4:5], ebuf[:, :],
                    mybir.AxisListType.X, mybir.AluOpType.add,
                ).then_inc(dve, 1)
                ve.wait_ge(dve, DVE_END + j * 4 + 2)
                ve.reciprocal(emx[:, 2:3], emx[:, 4:5]).then_inc(dve, 1)
                ve.wait_ge(dve, DVE_END + j * 4 + 3)
                ve.tensor_scalar_mul(
                    ebuf[:, :], ebuf[:, :], emx[:, 2:3]
                ).then_inc(dve, 1)

    nc.compile()
    return nc


def _prep_inputs(inputs, t_steps=T):
    bf = ml_dtypes.bfloat16
    images = np.asarray(inputs["images"], np.float32)
    captions = np.asarray(inputs["captions"])
    table = np.asarray(inputs["embed_table"], np.float32)
    W_ih = np.asarray(inputs["W_ih"], np.float32)
    W_hh = np.asarray(inputs["W_hh"], np.float32)
    W_hr = np.asarray(inputs["W_hr"], np.float32)
    bsum = (np.asarray(inputs["b_ih"], np.float32)
            + np.asarray(inputs["b_hh"], np.float32))

    P0, P1 = W_hr[0], W_hr[1]
    M00 = W_hh[0] @ P0
    M10 = W_ih[1] @ P0
    M11 = W_hh[1] @ P1

    emb = table[captions[:, :-1]]
    X = np.concatenate([images, emb], axis=1)  # [B, T, E]
    xT = np.ascontiguousarray(
        X.transpose(2, 1, 0)[:, :t_steps, :].reshape(E, t_steps * B)
    ).astype(bf)

    ones = np.zeros((128, 64), bf)
    ones[0, :] = 1
    iden = np.eye(64, dtype=np.float32).astype(bf)
    p1w = np.ascontiguousarray(P1.T.reshape(8, 128, 512)).astype(bf)

    in_maps = []
    for r in range(NCORES):
        # gate-column order per core: [i | f | o | g]  (jnp.split order i,f,g,o)
        rows = np.concatenate(
            [np.arange(g * 1024 + r * 128, g * 1024 + (r + 1) * 128)
             for g in (0, 1, 3, 2)]
        )
        w0 = np.zeros((13, 128, 512), bf)
        w0[0:4] = W_ih[0][rows].T.reshape(4, 128, 512).astype(bf)
        w0[4:12] = M00[rows].T.reshape(8, 128, 512).astype(bf)
        bt = np.zeros((128, 512), np.float32)
        bt[0, :] = bsum[0][rows]
        w0[12] = bt.astype(bf)
        w1 = np.zeros((17, 128, 512), bf)
        w1[0:8] = M10[rows].T.reshape(8, 128, 512).astype(bf)
        w1[8:16] = M11[rows].T.reshape(8, 128, 512).astype(bf)
        bt1 = np.zeros((128, 512), np.float32)
        bt1[0, :] = bsum[1][rows]
        w1[16] = bt1.astype(bf)
        in_maps.append({
            "w0": w0, "w1": w1, "p1w": p1w, "xT": xT,
            "ones": ones, "iden": iden,
            "rank": np.array([[r]], np.int32),
            "rank16": np.array([[r * (t_steps // NCORES)]], np.int32),
        })
    return in_maps


def kernel(**inputs):
    global LAST_EXEC_NS
    if TRACE:
        _install_trace_hook()
    if "nc" not in _CACHE:
        _CACHE["nc"] = build(T)
    nc = _CACHE["nc"]
    in_maps = _prep_inputs(inputs)
    res = run_bass_kernel_spmd(
        nc, in_maps, core_ids=list(range(8)), trace=TRACE
    )
    LAST_EXEC_NS = res.exec_time_ns
    out = np.concatenate([res.results[r]["y"] for r in range(8)], axis=1)
    return out.astype(np.float32)


if __name__ == "__main__":
    pass


def debug_run(inputs, t_steps=8):
    if TRACE:
        _install_trace_hook()
    nc = build(t_steps, dump=True)
    in_maps = _prep_inputs(inputs, t_steps)
    res = run_bass_kernel_spmd(nc, in_maps, core_ids=list(range(8)), trace=TRACE)
    y = np.concatenate([res.results[r]["y"] for r in range(8)], axis=1)
    s1d = [res.results[r]["s1store"] for r in range(8)]
    return y.astype(np.float32), s1d, res.exec_time_ns


# revision 14
# speedup vs baseline: 1.2689x; 1.2689x over previous
"""CNN-LSTM Trainium2 kernel: 8-way tensor-parallel over the 4H gate dim.

v2: split-exchange schedule.
- Host folds the hidden projection into the gate weights (M00 = W_hh0 @ W_hr0,
  M10 = W_ih1 @ W_hr0, M11 = W_hh1 @ W_hr1) so the recurrence runs entirely on
  the sharded s = sigmoid(o)*tanh(c) vectors (H=1024, 128 per core).
- Gate columns per core ordered [i | f | o | g] so one Sigmoid op covers
  i,f,o (384 cols) and one Tanh covers g.
- Two 16KB broadcasts per superstep: A carries s0^T(n) (triggered mid-superstep
  right after the L0 cell update), B carries s1^T(n) (triggered early superstep
  n+1).  The L0 recurrence cycle (arrival -> 8 MMs -> ACT/DVE -> transpose ->
  copy -> trigger -> flight) no longer waits on the L1 chain.
- Epilogue: h1 = P1 @ s1 + softmax, sharded over T (16 steps/core).
"""
import sys
import os
import numpy as np

sys.path.insert(0, "/opt/trn_rl_repo")

import concourse.bass as bass  # noqa: E402
import concourse.bacc as bacc  # noqa: E402
import concourse.mybir as mybir  # noqa: E402
from concourse.bass_utils import run_bass_kernel_spmd  # noqa: E402
import ml_dtypes  # noqa: E402

BF = mybir.dt.bfloat16
F32 = mybir.dt.float32
AF = mybir.ActivationFunctionType

B, T, E, H, V = 64, 128, 512, 1024, 10000
NCORES = 8
TRACE = False
LAST_EXEC_NS = None
_CACHE = {}


def _install_trace_hook():
    import types, contextlib, ctypes

    if "antenv.axon_hooks" in sys.modules:
        return
    mod = types.ModuleType("antenv.axon_hooks")
    mod._hook = None
    mod.set_axon_ntff_profile_hook = lambda h: setattr(mod, "_hook", h)
    mod.get_axon_ntff_profile_hook = lambda: mod._hook
    sys.modules["antenv.axon_hooks"] = mod
    import antenv

    antenv.axon_hooks = mod
    so_path = "/opt/axon/libaxon_pjrt.so"
    try:
        lib = ctypes.CDLL(so_path)
    except OSError:
        return
    if not hasattr(lib, "axon_start_nrt_profile"):
        return
    lib.axon_start_nrt_profile.argtypes = [ctypes.POINTER(ctypes.c_int64), ctypes.c_size_t]
    lib.axon_start_nrt_profile.restype = ctypes.c_int64
    lib.axon_stop_nrt_profile.argtypes = [ctypes.c_char_p]
    lib.axon_stop_nrt_profile.restype = ctypes.c_int64

    @contextlib.contextmanager
    def _hook(output_dir, device_ids):
        import jax

        jax.devices()
        if device_ids:
            ids = (ctypes.c_int64 * len(device_ids))(*device_ids)
            rc = lib.axon_start_nrt_profile(ids, len(device_ids))
        else:
            rc = lib.axon_start_nrt_profile(None, 0)
        if rc != 0:
            raise RuntimeError(f"axon_start_nrt_profile rc={rc}")
        try:
            yield
        finally:
            n = lib.axon_stop_nrt_profile(str(output_dir).encode())
            print(f"profile: {n} file(s) -> {output_dir}", file=sys.stderr)

    mod.set_axon_ntff_profile_hook(_hook)


def build(t_steps=T, dump=False):
    NS = t_steps + 3  # supersteps 0 .. t_steps+2
    TSH = t_steps // NCORES  # epilogue steps per core

    nc = bacc.Bacc("TRN2", target_bir_lowering=False, debug=False, num_devices=8)

    # ---- I/O ----
    w0d = nc.dram_tensor("w0", [13, 128, 512], BF, kind="ExternalInput")
    w1d = nc.dram_tensor("w1", [17, 128, 512], BF, kind="ExternalInput")
    p1d = nc.dram_tensor("p1w", [8, 128, 512], BF, kind="ExternalInput")
    xtd = nc.dram_tensor("xT", [512, t_steps * 64], BF, kind="ExternalInput")
    onesd = nc.dram_tensor("ones", [128, 64], BF, kind="ExternalInput")
    idend = nc.dram_tensor("iden", [64, 64], BF, kind="ExternalInput")
    rankd = nc.dram_tensor("rank", [1, 1], mybir.dt.int32, kind="ExternalInput")
    rank16d = nc.dram_tensor("rank16", [1, 1], mybir.dt.int32, kind="ExternalInput")
    yd = nc.dram_tensor("y", [64, TSH, 512], F32, kind="ExternalOutput")
    s1store = nc.dram_tensor(
        "s1store", [t_steps, 128 * 512], BF,
        kind="ExternalOutput" if dump else "Internal",
    )

    # ---- SBUF ----
    W0 = nc.alloc_sbuf_tensor("W0", [128, 13 * 512], BF)
    W1 = nc.alloc_sbuf_tensor("W1", [128, 17 * 512], BF)
    P1S = nc.alloc_sbuf_tensor("P1S", [128, 8 * 512], BF)
    Gb = [nc.alloc_sbuf_tensor(f"G{q}", [128, 1024], BF) for q in range(3)]
    SSA = [nc.alloc_sbuf_tensor(f"SSA{p}", [128, 64], BF) for p in range(2)]
    SSB = [nc.alloc_sbuf_tensor(f"SSB{p}", [128, 64], BF) for p in range(2)]
    XT = nc.alloc_sbuf_tensor("XT", [128, 2 * 256], BF)
    ONES = nc.alloc_sbuf_tensor("ONES", [128, 64], BF)
    IDN = nc.alloc_sbuf_tensor("IDN", [64, 64], BF)
    actb = nc.alloc_sbuf_tensor("actb", [64, 1024], F32)  # [i f o g] x 2 layers
    cbuf = nc.alloc_sbuf_tensor("cbuf", [64, 256], F32)  # c0 | c1
    thc = nc.alloc_sbuf_tensor("thc", [64, 256], F32)  # tanh(c0) | tanh(c1)
    sS = [nc.alloc_sbuf_tensor(f"sS{p}", [64, 256], BF) for p in range(2)]  # s0|s1
    es1 = [nc.alloc_sbuf_tensor(f"es1_{p}", [128, 512], BF) for p in range(2)]
    emx = nc.alloc_sbuf_tensor("emx", [64, 8], F32)  # max, negmax, sum, rsum slots
    ebuf = nc.alloc_sbuf_tensor("ebuf", [64, 512], F32)

    # ---- PSUM (8 banks total) ----
    ps_g0 = [nc.alloc_psum_tensor(f"psg0_{p}", [64, 512], F32) for p in range(2)]
    ps_g1 = [nc.alloc_psum_tensor(f"psg1_{p}", [64, 512], F32) for p in range(2)]
    ps_t = [nc.alloc_psum_tensor(f"pst_{p}", [128, 128], BF) for p in range(2)]
    ps_e = [nc.alloc_psum_tensor(f"pse_{p}", [64, 512], F32) for p in range(2)]

    # ---- semaphores ----
    rsA = [nc.alloc_semaphore(f"rsA{q}") for q in range(3)]
    rsB = [nc.alloc_semaphore(f"rsB{q}") for q in range(3)]
    prep = nc.alloc_semaphore("prep")
    lsemA = nc.alloc_semaphore("lsemA")
    lsemB = nc.alloc_semaphore("lsemB")
    pe = nc.alloc_semaphore("pe")
    acts = nc.alloc_semaphore("acts")
    dve = nc.alloc_semaphore("dve")
    xdma = nc.alloc_semaphore("xdma")
    sdma = nc.alloc_semaphore("sdma")
    edma = nc.alloc_semaphore("edma")
    idma = nc.alloc_semaphore("idma")
    init = nc.alloc_semaphore("init")
    ydma = nc.alloc_semaphore("ydma")

    rdests = [(0, k) for k in range(8)]

    # ---- schedule predicates ----
    def A_ex(m):
        return 0 <= m <= t_steps - 1

    def B_ex(m):
        return 2 <= m <= t_steps + 1

    def L0(n):
        return n <= t_steps - 1

    def D0(n):
        return 1 <= n <= t_steps - 1

    def L1(n):
        return 2 <= n <= t_steps + 1

    def B2(n):
        return 3 <= n <= t_steps + 1

    def TA(n):
        return A_ex(n)

    def TBp(n):  # transpose/copy/trigger slot for B(n-1)
        return B_ex(n - 1)

    def rthA(m):
        return 16 * (m // 3 + 1)

    def rthB(m):
        return 16 * ((m - 2) // 3 + 1)

    # ---- analytic milestone tables ----
    # PE program order per n: x(n)+bias, b1a(n), TB(n-1), b1b(n), D-g0(n),
    #                         b2(n)+bias, TA(n)
    pe_tb, pe_g0, pe_g1, pe_ta = {}, {}, {}, {}
    a_sig0, a_tg0, a_tc0, a_sig1, a_tg1, a_tc1 = {}, {}, {}, {}, {}, {}
    d_cpB, d_c0, d_s0, d_cpA, d_c1, d_s1 = {}, {}, {}, {}, {}, {}
    prep_A, prep_B = {}, {}
    c_pe = c_a = c_d = 0
    for n in range(NS):
        if L0(n) and not D0(n):  # n == 0: bias stop carries the inc
            c_pe += 1
        if not D0(n) and L0(n):
            pe_g0[n] = c_pe
        if TBp(n):
            c_pe += 1
        pe_tb[n] = c_pe
        if D0(n):
            c_pe += 1
            pe_g0[n] = c_pe
        if L1(n):
            c_pe += 1
        pe_g1[n] = c_pe
        if TA(n):
            c_pe += 1
        pe_ta[n] = c_pe

        if L0(n):
            c_a += 1
            a_sig0[n] = c_a
            c_a += 1
            a_tg0[n] = c_a
            c_a += 1
            a_tc0[n] = c_a
        if L1(n):
            c_a += 1
            a_sig1[n] = c_a
            c_a += 1
            a_tg1[n] = c_a
            c_a += 1
            a_tc1[n] = c_a

        if TBp(n):
            c_d += 1
        d_cpB[n] = c_d
        if L0(n):
            c_d += 1
            d_c0[n] = c_d
            c_d += 1
            d_s0[n] = c_d
        if TA(n):
            c_d += 1
        d_cpA[n] = c_d
        if L1(n):
            c_d += 1
            d_c1[n] = c_d
            c_d += 1
            d_s1[n] = c_d
    PE_END, ACT_END, DVE_END = c_pe, c_a, c_d

    # prep FIFO counter (A(0) prepped in prologue)
    c_p = 1
    prep_A[0] = 1
    for n in range(NS):
        if B_ex(n):
            c_p += 1
            prep_B[n] = c_p
        if A_ex(n + 1):
            c_p += 1
            prep_A[n + 1] = c_p

    # store count through B(m) (stores happen at superstep m+1)
    def st_cnt(m):  # number of s1store writes for B(2..m)
        return max(0, min(m, t_steps + 1) - 1) if m >= 2 else 0

    n_stores = st_cnt(t_steps + 1)  # == t_steps

    with nc.Block() as block:

        # ================= GPSIMD =================
        @block.gpsimd
        def _(g):
            with g.register("rank") as rank, g.register("urow") as urow, \
                    g.register("r16") as r16:
                g.load(rank, rankd.ap())
                g.load(r16, rank16d.ap())
                g.dma_start(
                    out=W0.rearrange("p (k c) -> p k c", k=13),
                    in_=w0d.rearrange("k p c -> p k c"),
                ).then_inc(idma, 16)
                g.dma_start(
                    out=W1.rearrange("p (k c) -> p k c", k=17),
                    in_=w1d.rearrange("k p c -> p k c"),
                ).then_inc(idma, 16)
                g.dma_start(
                    out=P1S.rearrange("p (k c) -> p k c", k=8),
                    in_=p1d.rearrange("k p c -> p k c"),
                ).then_inc(idma, 16)
                g.dma_start(out=ONES[:, :], in_=onesd[:, :]).then_inc(idma, 16)
                g.dma_start(out=IDN[:, :], in_=idend[:, :]).then_inc(idma, 16)
                g.wait_ge(idma, 80)
                g.memset(cbuf[:, :], 0.0)
                g.memset(emx[:, :], 0.0).then_inc(init, 1)
                g.bir_kernel_barrier_wait([list(range(8))])
                # prologue: prep A(0)
                for r in range(8):
                    with g.If_eq(rank, r):
                        g.remote_dma_broadcast(
                            out_ap=Gb[0][:, r * 128:r * 128 + 64],
                            in_ap=SSA[0][:, :],
                            remote_sem=rsA[0],
                            local_sem=lsemA,
                            rdests=rdests,
                        ).then_inc(prep, 1)
                for n in range(NS):
                    # --- B phase: trigger B(n-1), prep B(n) ---
                    if TBp(n):
                        g.wait_ge(dve, d_cpB[n])
                        g.wait_ge(prep, prep_B[n - 1])
                        if n >= 6:
                            g.wait_ge(sdma, 16 * st_cnt(n - 4))
                        g.trigger_dma(count=1)
                    if B_ex(n):
                        p3, p2 = n % 3, n % 2
                        for r in range(8):
                            with g.If_eq(rank, r):
                                g.remote_dma_broadcast(
                                    out_ap=Gb[p3][:, r * 128 + 64:(r + 1) * 128],
                                    in_ap=SSB[p2][:, :],
                                    remote_sem=rsB[(n - 2) % 3],
                                    local_sem=lsemB,
                                    rdests=rdests,
                                ).then_inc(prep, 1)
                    # --- A phase: trigger A(n), prep A(n+1) ---
                    if TA(n):
                        g.wait_ge(dve, d_cpA[n])
                        g.wait_ge(prep, prep_A[n])
                        g.trigger_dma(count=1)
                    if A_ex(n + 1):
                        p3, p2 = (n + 1) % 3, (n + 1) % 2
                        for r in range(8):
                            with g.If_eq(rank, r):
                                g.remote_dma_broadcast(
                                    out_ap=Gb[p3][:, r * 128:r * 128 + 64],
                                    in_ap=SSA[p2][:, :],
                                    remote_sem=rsA[p3],
                                    local_sem=lsemA,
                                    rdests=rdests,
                                ).then_inc(prep, 1)
                # ---- epilogue input DMAs ----
                g.wait_ge(sdma, 16 * n_stores)  # all s1 stores landed
                for j in range(TSH):
                    g.reg_add(urow, r16, j)
                    if j >= 2:
                        g.wait_ge(pe, PE_END + j - 1)  # es1[j%2] WAR
                    g.dma_start(
                        out=es1[j % 2][:, :],
                        in_=s1store[bass.ds(g.snap(urow), 1), :].rearrange(
                            "a (p c) -> (a p) c", p=128
                        ),
                    ).then_inc(edma, 16)

        # ================= SYNC (HWDGE staging/stores) =================
        @block.sync
        def _(sy):
            sy.wait_ge(init, 1)
            sy.dma_start(
                out=XT[:, 0:256].rearrange("p (a c) -> p a c", a=4),
                in_=xtd.rearrange("(a p) t -> p a t", p=128)[:, :, 0:64],
            ).then_inc(xdma, 16)
            for n in range(NS):
                if n + 1 <= t_steps - 1:
                    if n >= 1:
                        sy.wait_ge(pe, pe_g0[n - 1])
                    sy.dma_start(
                        out=XT[:, ((n + 1) % 2) * 256:((n + 1) % 2 + 1) * 256]
                        .rearrange("p (a c) -> p a c", a=4),
                        in_=xtd.rearrange("(a p) t -> p a t", p=128)[
                            :, :, (n + 1) * 64:(n + 2) * 64
                        ],
                    ).then_inc(xdma, 16)
                if B_ex(n - 1):  # store s1 of B(n-1) at superstep n
                    m = n - 1
                    sy.wait_ge(rsB[(m - 2) % 3], rthB(m))
                    sy.dma_start(
                        out=s1store[m - 2, :].rearrange(
                            "(p k c) -> p k c", p=128, k=8
                        ),
                        in_=Gb[m % 3].rearrange("p (k c) -> p k c", k=8)[
                            :, :, 64:128
                        ],
                    ).then_inc(sdma, 16)
            # epilogue output DMAs
            for j in range(TSH):
                sy.wait_ge(dve, DVE_END + j * 4 + 4)
                sy.dma_start(out=yd[:, j, :], in_=ebuf[:, :]).then_inc(ydma, 16)

        # ================= TENSOR (PE) =================
        @block.tensor
        def _(te):
            te.wait_ge(init, 1)
            for n in range(NS):
                p2, p3 = n % 2, n % 3
                # --- 1: g0 x-part + bias ---
                if L0(n):
                    te.wait_ge(xdma, 16 * (n + 1))
                    if n >= 2 and L0(n - 2):
                        te.wait_ge(acts, a_tg0[n - 2])  # ps_g0[p2] WAR
                    for k in range(4):
                        te.matmul(
                            ps_g0[p2][:, :],
                            XT[:, p2 * 256 + k * 64:p2 * 256 + (k + 1) * 64],
                            W0[:, k * 512:(k + 1) * 512],
                            start=(k == 0), stop=False,
                        )
                    mm = te.matmul(
                        ps_g0[p2][:, :], ONES[:, :], W0[:, 12 * 512:13 * 512],
                        start=False, stop=(not D0(n)),
                    )
                    if not D0(n):
                        mm.then_inc(pe, 1)
                # --- 2: g1 s0-part first chunk (exchange n-2) ---
                if L1(n):
                    te.wait_ge(rsA[(n - 2) % 3], rthA(n - 2))
                    if n >= 4 and L1(n - 2):
                        te.wait_ge(acts, a_tg1[n - 2])  # ps_g1[p2] WAR
                    for k in range(5):
                        te.matmul(
                            ps_g1[p2][:, :],
                            Gb[(n - 2) % 3][:, k * 128:k * 128 + 64],
                            W1[:, k * 512:(k + 1) * 512],
                            start=(k == 0), stop=False,
                        )
                # --- 3: transpose-B of s1(n-1) ---
                if TBp(n):
                    te.wait_ge(dve, d_s1[n - 1])
                    te.transpose(
                        ps_t[(n - 1) % 2][:, 64:128],
                        sS[(n - 1) % 2][:, 128:256], IDN[:, :],
                    ).then_inc(pe, 1)
                # --- 4: g1 s0-part rest ---
                if L1(n):
                    for k in range(5, 8):
                        te.matmul(
                            ps_g1[p2][:, :],
                            Gb[(n - 2) % 3][:, k * 128:k * 128 + 64],
                            W1[:, k * 512:(k + 1) * 512],
                            start=False, stop=False,
                        )
                # --- 5: D-g0 s-part (exchange n-1) ---
                if D0(n):
                    te.wait_ge(rsA[(n - 1) % 3], rthA(n - 1))
                    for k in range(8):
                        mm = te.matmul(
                            ps_g0[p2][:, :],
                            Gb[(n - 1) % 3][:, k * 128:k * 128 + 64],
                            W0[:, (4 + k) * 512:(5 + k) * 512],
                            start=False, stop=(k == 7),
                        )
                        if k == 7:
                            mm.then_inc(pe, 1)
                # --- 6: g1 s1-part (exchange B n-1) + bias ---
                if L1(n):
                    if B2(n):
                        te.wait_ge(rsB[(n - 3) % 3], rthB(n - 1))
                        for k in range(8):
                            te.matmul(
                                ps_g1[p2][:, :],
                                Gb[(n - 1) % 3][:, k * 128 + 64:(k + 1) * 128],
                                W1[:, (8 + k) * 512:(9 + k) * 512],
                                start=False, stop=False,
                            )
                    te.matmul(
                        ps_g1[p2][:, :], ONES[:, :], W1[:, 16 * 512:17 * 512],
                        start=False, stop=True,
                    ).then_inc(pe, 1)
                # --- 7: transpose-A of s0(n) ---
                if TA(n):
                    te.wait_ge(dve, d_s0[n])
                    te.transpose(
                        ps_t[p2][:, 0:64],
                        sS[p2][:, 0:128], IDN[:, :],
                    ).then_inc(pe, 1)
            # ---- epilogue GEMMs ----
            for j in range(TSH):
                te.wait_ge(edma, 16 * (j + 1))
                if j >= 2:
                    te.wait_ge(acts, ACT_END + j * 2 - 2)  # ps_e WAR
                mm_e = None
                for k in range(8):
                    mm_e = te.matmul(
                        ps_e[j % 2][:, :],
                        es1[j % 2][:, k * 64:(k + 1) * 64],
                        P1S[:, k * 512:(k + 1) * 512],
                        start=(k == 0), stop=(k == 7),
                    )
                mm_e.then_inc(pe, 1)

        # ================= SCALAR (ACT) =================
        @block.scalar
        def _(sc):
            for n in range(NS):
                p2 = n % 2
                if L0(n):
                    sc.wait_ge(pe, pe_g0[n])
                    sc.activation(
                        actb[:, 0:384], ps_g0[p2][:, 0:384], AF.Sigmoid
                    ).then_inc(acts, 1)
                    sc.activation(
                        actb[:, 384:512], ps_g0[p2][:, 384:512], AF.Tanh
                    ).then_inc(acts, 1)
                    sc.wait_ge(dve, d_c0[n])
                    sc.activation(
                        thc[:, 0:128], cbuf[:, 0:128], AF.Tanh
                    ).then_inc(acts, 1)
                if L1(n):
                    sc.wait_ge(pe, pe_g1[n])
                    sc.activation(
                        actb[:, 512:896], ps_g1[p2][:, 0:384], AF.Sigmoid
                    ).then_inc(acts, 1)
                    sc.activation(
                        actb[:, 896:1024], ps_g1[p2][:, 384:512], AF.Tanh
                    ).then_inc(acts, 1)
                    sc.wait_ge(dve, d_c1[n])
                    sc.activation(
                        thc[:, 128:256], cbuf[:, 128:256], AF.Tanh
                    ).then_inc(acts, 1)
            # epilogue: negmax + exp
            for j in range(TSH):
                sc.wait_ge(dve, DVE_END + j * 4 + 1)
                if j >= 1:
                    sc.wait_ge(ydma, 16 * j)  # ebuf WAR vs output DMA
                sc.activation(
                    emx[:, 1:2], emx[:, 0:1], AF.Copy, scale=-1.0
                ).then_inc(acts, 1)
                sc.wait_ge(acts, ACT_END + j * 2 + 1)
                sc.activation(
                    ebuf[:, :], ps_e[j % 2][:, :], AF.Exp, bias=emx[:, 1:2]
                ).then_inc(acts, 1)

        # ================= VECTOR (DVE) =================
        @block.vector
        def _(ve):
            for n in range(NS):
                p2 = n % 2
                # copy-B of s1(n-1)
                if TBp(n):
                    ve.wait_ge(pe, pe_tb[n])
                    if n >= 5:
                        ve.wait_ge(lsemB, 16 * (n - 4))  # SSB[(n-1)%2] WAR
                    ve.tensor_copy(
                        SSB[(n - 1) % 2][:, :], ps_t[(n - 1) % 2][:, 64:128]
                    ).then_inc(dve, 1)
                # L0 cell update
                if L0(n):
                    ve.wait_ge(acts, a_sig0[n])
                    ve.tensor_mul(cbuf[:, 0:128], actb[:, 128:256], cbuf[:, 0:128])
                    ve.wait_ge(acts, a_tg0[n])
                    ve.tensor_mul(actb[:, 0:128], actb[:, 0:128], actb[:, 384:512])
                    ve.tensor_add(
                        cbuf[:, 0:128], cbuf[:, 0:128], actb[:, 0:128]
                    ).then_inc(dve, 1)
                    ve.wait_ge(acts, a_tc0[n])
                    ve.tensor_mul(
                        sS[p2][:, 0:128], actb[:, 256:384], thc[:, 0:128]
                    ).then_inc(dve, 1)
                # copy-A of s0(n)
                if TA(n):
                    ve.wait_ge(pe, pe_ta[n])
                    if n >= 2:
                        ve.wait_ge(lsemA, 16 * (n - 1))  # SSA[p2] WAR
                    ve.tensor_copy(
                        SSA[p2][:, :], ps_t[p2][:, 0:64]
                    ).then_inc(dve, 1)
                # L1 cell update
                if L1(n):
                    ve.wait_ge(acts, a_sig1[n])
                    ve.tensor_mul(cbuf[:, 128:256], actb[:, 640:768], cbuf[:, 128:256])
                    ve.wait_ge(acts, a_tg1[n])
                    ve.tensor_mul(actb[:, 512:640], actb[:, 512:640], actb[:, 896:1024])
                    ve.tensor_add(
                        cbuf[:, 128:256], cbuf[:, 128:256], actb[:, 512:640]
                    ).then_inc(dve, 1)
                    ve.wait_ge(acts, a_tc1[n])
                    ve.tensor_mul(
                        sS[p2][:, 128:256], actb[:, 768:896], thc[:, 128:256]
                    ).then_inc(dve, 1)
            # epilogue: max, sum, scale
            for j in range(TSH):
                ve.wait_ge(pe, PE_END + j + 1)
                if j >= 1:
                    ve.wait_ge(acts, ACT_END + j * 2 - 1)
                ve.tensor_reduce(
                    emx[:, 0:1], ps_e[j % 2][:, :],
                    mybir.AxisListType.X, mybir.AluOpType.max,
                ).then_inc(dve, 1)
                ve.wait_ge(acts, ACT_END + j * 2 + 2)
                ve.tensor_reduce(
                    emx[:, 4:5], ebuf[:, :],
                    mybir.AxisListType.X, mybir.AluOpType.add,
                ).then_inc(dve, 1)
                ve.wait_ge(dve, DVE_END + j * 4 + 2)
                ve.reciprocal(emx[:, 2:3], emx[:, 4:5]).then_inc(dve, 1)
                ve.wait_ge(dve, DVE_END + j * 4 + 3)
                ve.tensor_scalar_mul(
                    ebuf[:, :], ebuf[:, :], emx[:, 2:3]
                ).then_inc(dve, 1)

    nc.compile()
    return nc


def _prep_inputs(inputs, t_steps=T):
    bf = ml_dtypes.bfloat16
    images = np.asarray(inputs["images"], np.float32)
    captions = np.asarray(inputs["captions"])
    table = np.asarray(inputs["embed_table"], np.float32)
    W_ih = np.asarray(inputs["W_ih"], np.float32)
    W_hh = np.asarray(inputs["W_hh"], np.float32)
    W_hr = np.asarray(inputs["W_hr"], np.float32)
    bsum = (np.asarray(inputs["b_ih"], np.float32)
            + np.asarray(inputs["b_hh"], np.float32))

    P0, P1 = W_hr[0], W_hr[1]
    M00 = W_hh[0] @ P0
    M10 = W_ih[1] @ P0
    M11 = W_hh[1] @ P1

    emb = table[captions[:, :-1]]
    X = np.concatenate([images, emb], axis=1)  # [B, T, E]
    xT = np.ascontiguousarray(
        X.transpose(2, 1, 0)[:, :t_steps, :].reshape(E, t_steps * B)
    ).astype(bf)

    ones = np.zeros((128, 64), bf)
    ones[0, :] = 1
    iden = np.eye(64, dtype=np.float32).astype(bf)
    p1w = np.ascontiguousarray(P1.T.reshape(8, 128, 512)).astype(bf)

    in_maps = []
    for r in range(NCORES):
        # gate-column order per core: [i | f | o | g]  (jnp.split order i,f,g,o)
        rows = np.concatenate(
            [np.arange(g * 1024 + r * 128, g * 1024 + (r + 1) * 128)
             for g in (0, 1, 3, 2)]
        )
        w0 = np.zeros((13, 128, 512), bf)
        w0[0:4] = W_ih[0][rows].T.reshape(4, 128, 512).astype(bf)
        w0[4:12] = M00[rows].T.reshape(8, 128, 512).astype(bf)
        bt = np.zeros((128, 512), np.float32)
        bt[0, :] = bsum[0][rows]
        w0[12] = bt.astype(bf)
        w1 = np.zeros((17, 128, 512), bf)
        w1[0:8] = M10[rows].T.reshape(8, 128, 512).astype(bf)
        w1[8:16] = M11[rows].T.reshape(8, 128, 512).astype(bf)
        bt1 = np.zeros((128, 512), np.float32)
        bt1[0, :] = bsum[1][rows]
        w1[16] = bt1.astype(bf)
        in_maps.append({
            "w0": w0, "w1": w1, "p1w": p1w, "xT": xT,
            "ones": ones, "iden": iden,
            "rank": np.array([[r]], np.int32),
            "rank16": np.array([[r * (t_steps // NCORES)]], np.int32),
        })
    return in_maps


def kernel(**inputs):
    global LAST_EXEC_NS
    if TRACE:
        _install_trace_hook()
    if "nc" not in _CACHE:
        _CACHE["nc"] = build(T)
    nc = _CACHE["nc"]
    in_maps = _prep_inputs(inputs)
    res = run_bass_kernel_spmd(
        nc, in_maps, core_ids=list(range(8)), trace=TRACE
    )
    LAST_EXEC_NS = res.exec_time_ns
    out = np.concatenate([res.results[r]["y"] for r in range(8)], axis=1)
    return out.astype(np.float32)


if __name__ == "__main__":
    pass


def debug_run(inputs, t_steps=8):
    if TRACE:
        _install_trace_hook()
    nc = build(t_steps, dump=True)
    in_maps = _prep_inputs(inputs, t_steps)
    res = run_bass_kernel_spmd(nc, in_maps, core_ids=list(range(8)), trace=TRACE)
    y = np.concatenate([res.results[r]["y"] for r in range(8)], axis=1)
    s1d = [res.results[r]["s1store"] for r in range(8)]
    return y.astype(np.float32), s1d, res.exec_time_ns


# revision 15
# speedup vs baseline: 1.2727x; 1.0030x over previous
"""CNN-LSTM Trainium2 kernel: 8-way tensor-parallel over the 4H gate dim.

v2: split-exchange schedule.
- Host folds the hidden projection into the gate weights (M00 = W_hh0 @ W_hr0,
  M10 = W_ih1 @ W_hr0, M11 = W_hh1 @ W_hr1) so the recurrence runs entirely on
  the sharded s = sigmoid(o)*tanh(c) vectors (H=1024, 128 per core).
- Gate columns per core ordered [i | f | o | g] so one Sigmoid op covers
  i,f,o (384 cols) and one Tanh covers g.
- Two 16KB broadcasts per superstep: A carries s0^T(n) (triggered mid-superstep
  right after the L0 cell update), B carries s1^T(n) (triggered early superstep
  n+1).  The L0 recurrence cycle (arrival -> 8 MMs -> ACT/DVE -> transpose ->
  copy -> trigger -> flight) no longer waits on the L1 chain.
- Epilogue: h1 = P1 @ s1 + softmax, sharded over T (16 steps/core).
"""
import sys
import os
import numpy as np

sys.path.insert(0, "/opt/trn_rl_repo")

import concourse.bass as bass  # noqa: E402
import concourse.bacc as bacc  # noqa: E402
import concourse.mybir as mybir  # noqa: E402
from concourse.bass_utils import run_bass_kernel_spmd  # noqa: E402
import ml_dtypes  # noqa: E402

BF = mybir.dt.bfloat16
F32 = mybir.dt.float32
AF = mybir.ActivationFunctionType

B, T, E, H, V = 64, 128, 512, 1024, 10000
NCORES = 8
TRACE = False
LAST_EXEC_NS = None
_CACHE = {}


def _install_trace_hook():
    import types, contextlib, ctypes

    if "antenv.axon_hooks" in sys.modules:
        return
    mod = types.ModuleType("antenv.axon_hooks")
    mod._hook = None
    mod.set_axon_ntff_profile_hook = lambda h: setattr(mod, "_hook", h)
    mod.get_axon_ntff_profile_hook = lambda: mod._hook
    sys.modules["antenv.axon_hooks"] = mod
    import antenv

    antenv.axon_hooks = mod
    so_path = "/opt/axon/libaxon_pjrt.so"
    try:
        lib = ctypes.CDLL(so_path)
    except OSError:
        return
    if not hasattr(lib, "axon_start_nrt_profile"):
        return
    lib.axon_start_nrt_profile.argtypes = [ctypes.POINTER(ctypes.c_int64), ctypes.c_size_t]
    lib.axon_start_nrt_profile.restype = ctypes.c_int64
    lib.axon_stop_nrt_profile.argtypes = [ctypes.c_char_p]
    lib.axon_stop_nrt_profile.restype = ctypes.c_int64

    @contextlib.contextmanager
    def _hook(output_dir, device_ids):
        import jax

        jax.devices()
        if device_ids:
            ids = (ctypes.c_int64 * len(device_ids))(*device_ids)
            rc = lib.axon_start_nrt_profile(ids, len(device_ids))
        else:
            rc = lib.axon_start_nrt_profile(None, 0)
        if rc != 0:
            raise RuntimeError(f"axon_start_nrt_profile rc={rc}")
        try:
            yield
        finally:
            n = lib.axon_stop_nrt_profile(str(output_dir).encode())
            print(f"profile: {n} file(s) -> {output_dir}", file=sys.stderr)

    mod.set_axon_ntff_profile_hook(_hook)


def build(t_steps=T, dump=False):
    NS = t_steps + 3  # supersteps 0 .. t_steps+2
    TSH = t_steps // NCORES  # epilogue steps per core

    nc = bacc.Bacc("TRN2", target_bir_lowering=False, debug=False, num_devices=8)

    # ---- I/O ----
    w0d = nc.dram_tensor("w0", [13, 128, 512], BF, kind="ExternalInput")
    w1d = nc.dram_tensor("w1", [17, 128, 512], BF, kind="ExternalInput")
    p1d = nc.dram_tensor("p1w", [8, 128, 512], BF, kind="ExternalInput")
    xtd = nc.dram_tensor("xT", [512, t_steps * 64], BF, kind="ExternalInput")
    onesd = nc.dram_tensor("ones", [128, 64], BF, kind="ExternalInput")
    idend = nc.dram_tensor("iden", [64, 64], BF, kind="ExternalInput")
    rankd = nc.dram_tensor("rank", [1, 1], mybir.dt.int32, kind="ExternalInput")
    rank16d = nc.dram_tensor("rank16", [1, 1], mybir.dt.int32, kind="ExternalInput")
    yd = nc.dram_tensor("y", [64, TSH, 512], F32, kind="ExternalOutput")
    s1store = nc.dram_tensor(
        "s1store", [t_steps, 128 * 512], BF,
        kind="ExternalOutput" if dump else "Internal",
    )

    # ---- SBUF ----
    W0 = nc.alloc_sbuf_tensor("W0", [128, 13 * 512], BF)
    W1 = nc.alloc_sbuf_tensor("W1", [128, 17 * 512], BF)
    P1S = nc.alloc_sbuf_tensor("P1S", [128, 8 * 512], BF)
    Gb = [nc.alloc_sbuf_tensor(f"G{q}", [128, 1024], BF) for q in range(3)]
    SSA = [nc.alloc_sbuf_tensor(f"SSA{p}", [128, 64], BF) for p in range(2)]
    SSB = [nc.alloc_sbuf_tensor(f"SSB{p}", [128, 64], BF) for p in range(2)]
    XT = nc.alloc_sbuf_tensor("XT", [128, 2 * 256], BF)
    ONES = nc.alloc_sbuf_tensor("ONES", [128, 64], BF)
    IDN = nc.alloc_sbuf_tensor("IDN", [64, 64], BF)
    actb = nc.alloc_sbuf_tensor("actb", [64, 1024], F32)  # [i f o g] x 2 layers
    cbuf = nc.alloc_sbuf_tensor("cbuf", [64, 256], F32)  # c0 | c1
    thc = nc.alloc_sbuf_tensor("thc", [64, 256], F32)  # tanh(c0) | tanh(c1)
    sS = [nc.alloc_sbuf_tensor(f"sS{p}", [64, 256], BF) for p in range(2)]  # s0|s1
    es1 = [nc.alloc_sbuf_tensor(f"es1_{p}", [128, 512], BF) for p in range(2)]
    emx = nc.alloc_sbuf_tensor("emx", [64, 8], F32)  # max, negmax, sum, rsum slots
    ebuf = nc.alloc_sbuf_tensor("ebuf", [64, 512], F32)

    # ---- PSUM (8 banks total) ----
    ps_g0 = [nc.alloc_psum_tensor(f"psg0_{p}", [64, 512], F32) for p in range(2)]
    ps_g1 = [nc.alloc_psum_tensor(f"psg1_{p}", [64, 512], F32) for p in range(2)]
    ps_t = [nc.alloc_psum_tensor(f"pst_{p}", [128, 128], BF) for p in range(2)]
    ps_e = [nc.alloc_psum_tensor(f"pse_{p}", [64, 512], F32) for p in range(2)]

    # ---- semaphores ----
    rsA = [nc.alloc_semaphore(f"rsA{q}") for q in range(3)]
    rsB = [nc.alloc_semaphore(f"rsB{q}") for q in range(3)]
    prep = nc.alloc_semaphore("prep")
    lsemA = nc.alloc_semaphore("lsemA")
    lsemB = nc.alloc_semaphore("lsemB")
    pe = nc.alloc_semaphore("pe")
    acts = nc.alloc_semaphore("acts")
    dve = nc.alloc_semaphore("dve")
    xdma = nc.alloc_semaphore("xdma")
    sdma = nc.alloc_semaphore("sdma")
    edma = nc.alloc_semaphore("edma")
    idma = nc.alloc_semaphore("idma")
    init = nc.alloc_semaphore("init")
    ydma = nc.alloc_semaphore("ydma")

    rdests = [(0, k) for k in range(8)]

    # ---- schedule predicates ----
    def A_ex(m):
        return 0 <= m <= t_steps - 1

    def B_ex(m):
        return 2 <= m <= t_steps + 1

    def L0(n):
        return n <= t_steps - 1

    def D0(n):
        return 1 <= n <= t_steps - 1

    def L1(n):
        return 2 <= n <= t_steps + 1

    def B2(n):
        return 3 <= n <= t_steps + 1

    def TA(n):
        return A_ex(n)

    def TBp(n):  # transpose/copy/trigger slot for B(n-1)
        return B_ex(n - 1)

    def rthA(m):
        return 16 * (m // 3 + 1)

    def rthB(m):
        return 16 * ((m - 2) // 3 + 1)

    # ---- analytic milestone tables ----
    # PE program order per n: x(n)+bias, b1a(n), TB(n-1), b1b(n), D-g0(n),
    #                         b2(n)+bias, TA(n)
    pe_tb, pe_g0, pe_g1, pe_ta = {}, {}, {}, {}
    a_sig0, a_tg0, a_tc0, a_sig1, a_tg1, a_tc1 = {}, {}, {}, {}, {}, {}
    d_cpB, d_c0, d_s0, d_cpA, d_c1, d_s1 = {}, {}, {}, {}, {}, {}
    prep_A, prep_B = {}, {}
    c_pe = c_a = c_d = 0
    for n in range(NS):
        if L0(n) and not D0(n):  # n == 0: bias stop carries the inc
            c_pe += 1
        if not D0(n) and L0(n):
            pe_g0[n] = c_pe
        if TBp(n):
            c_pe += 1
        pe_tb[n] = c_pe
        if D0(n):
            c_pe += 1
            pe_g0[n] = c_pe
        if L1(n):
            c_pe += 1
        pe_g1[n] = c_pe
        if TA(n):
            c_pe += 1
        pe_ta[n] = c_pe

        if L0(n):
            c_a += 1
            a_sig0[n] = c_a
            c_a += 1
            a_tg0[n] = c_a
            c_a += 1
            a_tc0[n] = c_a
        if L1(n):
            c_a += 1
            a_sig1[n] = c_a
            c_a += 1
            a_tg1[n] = c_a
            c_a += 1
            a_tc1[n] = c_a

        if TBp(n):
            c_d += 1
        d_cpB[n] = c_d
        if L0(n):
            c_d += 1
            d_c0[n] = c_d
            c_d += 1
            d_s0[n] = c_d
        if TA(n):
            c_d += 1
        d_cpA[n] = c_d
        if L1(n):
            c_d += 1
            d_c1[n] = c_d
            c_d += 1
            d_s1[n] = c_d
    PE_END, ACT_END, DVE_END = c_pe, c_a, c_d

    # prep FIFO counter (A(0) prepped in prologue)
    c_p = 1
    prep_A[0] = 1
    for n in range(NS):
        if B_ex(n):
            c_p += 1
            prep_B[n] = c_p
        if A_ex(n + 1):
            c_p += 1
            prep_A[n + 1] = c_p

    # store count through B(m) (stores happen at superstep m+1)
    def st_cnt(m):  # number of s1store writes for B(2..m)
        return max(0, min(m, t_steps + 1) - 1) if m >= 2 else 0

    n_stores = st_cnt(t_steps + 1)  # == t_steps

    with nc.Block() as block:

        # ================= GPSIMD =================
        @block.gpsimd
        def _(g):
            with g.register("rank") as rank, g.register("urow") as urow, \
                    g.register("r16") as r16:
                g.load(rank, rankd.ap())
                g.load(r16, rank16d.ap())
                g.dma_start(
                    out=W0.rearrange("p (k c) -> p k c", k=13),
                    in_=w0d.rearrange("k p c -> p k c"),
                ).then_inc(idma, 16)
                g.dma_start(
                    out=W1.rearrange("p (k c) -> p k c", k=17),
                    in_=w1d.rearrange("k p c -> p k c"),
                ).then_inc(idma, 16)
                g.dma_start(
                    out=P1S.rearrange("p (k c) -> p k c", k=8),
                    in_=p1d.rearrange("k p c -> p k c"),
                ).then_inc(idma, 16)
                g.dma_start(out=ONES[:, :], in_=onesd[:, :]).then_inc(idma, 16)
                g.dma_start(out=IDN[:, :], in_=idend[:, :]).then_inc(idma, 16)
                g.wait_ge(idma, 80)
                g.memset(cbuf[:, :], 0.0)
                g.memset(emx[:, :], 0.0).then_inc(init, 1)
                g.bir_kernel_barrier_wait([list(range(8))])
                # prologue: prep A(0)
                for r in range(8):
                    with g.If_eq(rank, r):
                        g.remote_dma_broadcast(
                            out_ap=Gb[0][:, r * 128:r * 128 + 64],
                            in_ap=SSA[0][:, :],
                            remote_sem=rsA[0],
                            local_sem=lsemA,
                            rdests=rdests,
                        ).then_inc(prep, 1)
                for n in range(NS):
                    # --- B phase: trigger B(n-1), prep B(n) ---
                    if TBp(n):
                        g.wait_ge(dve, d_cpB[n])
                        g.wait_ge(prep, prep_B[n - 1])
                        if n >= 6:
                            g.wait_ge(sdma, 16 * st_cnt(n - 4))
                        g.trigger_dma(count=1)
                    if B_ex(n):
                        p3, p2 = n % 3, n % 2
                        for r in range(8):
                            with g.If_eq(rank, r):
                                g.remote_dma_broadcast(
                                    out_ap=Gb[p3][:, r * 128 + 64:(r + 1) * 128],
                                    in_ap=SSB[p2][:, :],
                                    remote_sem=rsB[(n - 2) % 3],
                                    local_sem=lsemB,
                                    rdests=rdests,
                                ).then_inc(prep, 1)
                    # --- A phase: trigger A(n), prep A(n+1) ---
                    if TA(n):
                        g.wait_ge(dve, d_cpA[n])
                        g.wait_ge(prep, prep_A[n])
                        g.trigger_dma(count=1)
                    if A_ex(n + 1):
                        p3, p2 = (n + 1) % 3, (n + 1) % 2
                        for r in range(8):
                            with g.If_eq(rank, r):
                                g.remote_dma_broadcast(
                                    out_ap=Gb[p3][:, r * 128:r * 128 + 64],
                                    in_ap=SSA[p2][:, :],
                                    remote_sem=rsA[p3],
                                    local_sem=lsemA,
                                    rdests=rdests,
                                ).then_inc(prep, 1)
                # ---- epilogue input DMAs ----
                g.wait_ge(sdma, 16 * n_stores)  # all s1 stores landed
                for j in range(TSH):
                    g.reg_add(urow, r16, j)
                    if j >= 2:
                        g.wait_ge(pe, PE_END + j - 1)  # es1[j%2] WAR
                    g.dma_start(
                        out=es1[j % 2][:, :],
                        in_=s1store[bass.ds(g.snap(urow), 1), :].rearrange(
                            "a (p c) -> (a p) c", p=128
                        ),
                    ).then_inc(edma, 16)

        # ================= SYNC (HWDGE staging/stores) =================
        @block.sync
        def _(sy):
            sy.wait_ge(init, 1)
            sy.dma_start(
                out=XT[:, 0:256].rearrange("p (a c) -> p a c", a=4),
                in_=xtd.rearrange("(a p) t -> p a t", p=128)[:, :, 0:64],
            ).then_inc(xdma, 16)
            for n in range(NS):
                if n + 1 <= t_steps - 1:
                    if n >= 1:
                        sy.wait_ge(pe, pe_g0[n - 1])
                    sy.dma_start(
                        out=XT[:, ((n + 1) % 2) * 256:((n + 1) % 2 + 1) * 256]
                        .rearrange("p (a c) -> p a c", a=4),
                        in_=xtd.rearrange("(a p) t -> p a t", p=128)[
                            :, :, (n + 1) * 64:(n + 2) * 64
                        ],
                    ).then_inc(xdma, 16)
                if B_ex(n - 1):  # store s1 of B(n-1) at superstep n
                    m = n - 1
                    sy.wait_ge(rsB[(m - 2) % 3], rthB(m))
                    sy.dma_start(
                        out=s1store[m - 2, :].rearrange(
                            "(p k c) -> p k c", p=128, k=8
                        ),
                        in_=Gb[m % 3].rearrange("p (k c) -> p k c", k=8)[
                            :, :, 64:128
                        ],
                    ).then_inc(sdma, 16)
            # epilogue output DMAs
            for j in range(TSH):
                sy.wait_ge(dve, DVE_END + j * 4 + 4)
                sy.dma_start(out=yd[:, j, :], in_=ebuf[:, :]).then_inc(ydma, 16)

        # ================= TENSOR (PE) =================
        @block.tensor
        def _(te):
            te.wait_ge(init, 1)
            for n in range(NS):
                p2, p3 = n % 2, n % 3
                # --- 1: g0 x-part + bias ---
                if L0(n):
                    te.wait_ge(xdma, 16 * (n + 1))
                    if n >= 2 and L0(n - 2):
                        te.wait_ge(acts, a_tg0[n - 2])  # ps_g0[p2] WAR
                    for k in range(4):
                        te.matmul(
                            ps_g0[p2][:, :],
                            XT[:, p2 * 256 + k * 64:p2 * 256 + (k + 1) * 64],
                            W0[:, k * 512:(k + 1) * 512],
                            start=(k == 0), stop=False,
                        )
                    mm = te.matmul(
                        ps_g0[p2][:, :], ONES[:, :], W0[:, 12 * 512:13 * 512],
                        start=False, stop=(not D0(n)),
                    )
                    if not D0(n):
                        mm.then_inc(pe, 1)
                # --- 2: transpose-B of s1(n-1) (early -> early trigger-B) ---
                if TBp(n):
                    te.wait_ge(dve, d_s1[n - 1])
                    te.transpose(
                        ps_t[(n - 1) % 2][:, 64:128],
                        sS[(n - 1) % 2][:, 128:256], IDN[:, :],
                    ).then_inc(pe, 1)
                # --- 3: g1 s0-part (exchange n-2) ---
                if L1(n):
                    te.wait_ge(rsA[(n - 2) % 3], rthA(n - 2))
                    if n >= 4 and L1(n - 2):
                        te.wait_ge(acts, a_tg1[n - 2])  # ps_g1[p2] WAR
                    for k in range(8):
                        te.matmul(
                            ps_g1[p2][:, :],
                            Gb[(n - 2) % 3][:, k * 128:k * 128 + 64],
                            W1[:, k * 512:(k + 1) * 512],
                            start=(k == 0), stop=False,
                        )
                # --- 5: D-g0 s-part (exchange n-1) ---
                if D0(n):
                    te.wait_ge(rsA[(n - 1) % 3], rthA(n - 1))
                    for k in range(8):
                        mm = te.matmul(
                            ps_g0[p2][:, :],
                            Gb[(n - 1) % 3][:, k * 128:k * 128 + 64],
                            W0[:, (4 + k) * 512:(5 + k) * 512],
                            start=False, stop=(k == 7),
                        )
                        if k == 7:
                            mm.then_inc(pe, 1)
                # --- 6: g1 s1-part (exchange B n-1) + bias ---
                if L1(n):
                    if B2(n):
                        te.wait_ge(rsB[(n - 3) % 3], rthB(n - 1))
                        for k in range(8):
                            te.matmul(
                                ps_g1[p2][:, :],
                                Gb[(n - 1) % 3][:, k * 128 + 64:(k + 1) * 128],
                                W1[:, (8 + k) * 512:(9 + k) * 512],
                                start=False, stop=False,
                            )
                    te.matmul(
                        ps_g1[p2][:, :], ONES[:, :], W1[:, 16 * 512:17 * 512],
                        start=False, stop=True,
                    ).then_inc(pe, 1)
                # --- 7: transpose-A of s0(n) ---
                if TA(n):
                    te.wait_ge(dve, d_s0[n])
                    te.transpose(
                        ps_t[p2][:, 0:64],
                        sS[p2][:, 0:128], IDN[:, :],
                    ).then_inc(pe, 1)
            # ---- epilogue GEMMs ----
            for j in range(TSH):
                te.wait_ge(edma, 16 * (j + 1))
                if j >= 2:
                    te.wait_ge(acts, ACT_END + j * 2 - 2)  # ps_e WAR
                mm_e = None
                for k in range(8):
                    mm_e = te.matmul(
                        ps_e[j % 2][:, :],
                        es1[j % 2][:, k * 64:(k + 1) * 64],
                        P1S[:, k * 512:(k + 1) * 512],
                        start=(k == 0), stop=(k == 7),
                    )
                mm_e.then_inc(pe, 1)

        # ================= SCALAR (ACT) =================
        @block.scalar
        def _(sc):
            for n in range(NS):
                p2 = n % 2
                if L0(n):
                    sc.wait_ge(pe, pe_g0[n])
                    sc.activation(
                        actb[:, 0:384], ps_g0[p2][:, 0:384], AF.Sigmoid
                    ).then_inc(acts, 1)
                    sc.activation(
                        actb[:, 384:512], ps_g0[p2][:, 384:512], AF.Tanh
                    ).then_inc(acts, 1)
                    sc.wait_ge(dve, d_c0[n])
                    sc.activation(
                        thc[:, 0:128], cbuf[:, 0:128], AF.Tanh
                    ).then_inc(acts, 1)
                if L1(n):
                    sc.wait_ge(pe, pe_g1[n])
                    sc.activation(
                        actb[:, 512:896], ps_g1[p2][:, 0:384], AF.Sigmoid
                    ).then_inc(acts, 1)
                    sc.activation(
                        actb[:, 896:1024], ps_g1[p2][:, 384:512], AF.Tanh
                    ).then_inc(acts, 1)
                    sc.wait_ge(dve, d_c1[n])
                    sc.activation(
                        thc[:, 128:256], cbuf[:, 128:256], AF.Tanh
                    ).then_inc(acts, 1)
            # epilogue: negmax + exp
            for j in range(TSH):
                sc.wait_ge(dve, DVE_END + j * 4 + 1)
                if j >= 1:
                    sc.wait_ge(ydma, 16 * j)  # ebuf WAR vs output DMA
                sc.activation(
                    emx[:, 1:2], emx[:, 0:1], AF.Copy, scale=-1.0
                ).then_inc(acts, 1)
                sc.wait_ge(acts, ACT_END + j * 2 + 1)
                sc.activation(
                    ebuf[:, :], ps_e[j % 2][:, :], AF.Exp, bias=emx[:, 1:2]
                ).then_inc(acts, 1)

        # ================= VECTOR (DVE) =================
        @block.vector
        def _(ve):
            for n in range(NS):
                p2 = n % 2
                # copy-B of s1(n-1)
                if TBp(n):
                    ve.wait_ge(pe, pe_tb[n])
                    if n >= 5:
                        ve.wait_ge(lsemB, 16 * (n - 4))  # SSB[(n-1)%2] WAR
                    ve.tensor_copy(
                        SSB[(n - 1) % 2][:, :], ps_t[(n - 1) % 2][:, 64:128]
                    ).then_inc(dve, 1)
                # L0 cell update
                if L0(n):
                    ve.wait_ge(acts, a_sig0[n])
                    ve.tensor_mul(cbuf[:, 0:128], actb[:, 128:256], cbuf[:, 0:128])
                    ve.wait_ge(acts, a_tg0[n])
                    ve.tensor_mul(actb[:, 0:128], actb[:, 0:128], actb[:, 384:512])
                    ve.tensor_add(
                        cbuf[:, 0:128], cbuf[:, 0:128], actb[:, 0:128]
                    ).then_inc(dve, 1)
                    ve.wait_ge(acts, a_tc0[n])
                    ve.tensor_mul(
                        sS[p2][:, 0:128], actb[:, 256:384], thc[:, 0:128]
                    ).then_inc(dve, 1)
                # copy-A of s0(n)
                if TA(n):
                    ve.wait_ge(pe, pe_ta[n])
                    if n >= 2:
                        ve.wait_ge(lsemA, 16 * (n - 1))  # SSA[p2] WAR
                    ve.tensor_copy(
                        SSA[p2][:, :], ps_t[p2][:, 0:64]
                    ).then_inc(dve, 1)
                # L1 cell update
                if L1(n):
                    ve.wait_ge(acts, a_sig1[n])
                    ve.tensor_mul(cbuf[:, 128:256], actb[:, 640:768], cbuf[:, 128:256])
                    ve.wait_ge(acts, a_tg1[n])
                    ve.tensor_mul(actb[:, 512:640], actb[:, 512:640], actb[:, 896:1024])
                    ve.tensor_add(
                        cbuf[:, 128:256], cbuf[:, 128:256], actb[:, 512:640]
                    ).then_inc(dve, 1)
                    ve.wait_ge(acts, a_tc1[n])
                    ve.tensor_mul(
                        sS[p2][:, 128:256], actb[:, 768:896], thc[:, 128:256]
                    ).then_inc(dve, 1)
            # epilogue: max, sum, scale
            for j in range(TSH):
                ve.wait_ge(pe, PE_END + j + 1)
                if j >= 1:
                    ve.wait_ge(acts, ACT_END + j * 2 - 1)
                ve.tensor_reduce(
                    emx[:, 0:1], ps_e[j % 2][:, :],
                    mybir.AxisListType.X, mybir.AluOpType.max,
                ).then_inc(dve, 1)
                ve.wait_ge(acts, ACT_END + j * 2 + 2)
                ve.tensor_reduce(
                    emx[:, 4:5], ebuf[:, :],
                    mybir.AxisListType.X, mybir.AluOpType.add,
                ).then_inc(dve, 1)
                ve.wait_ge(dve, DVE_END + j * 4 + 2)
                ve.reciprocal(emx[:, 2:3], emx[:, 4:5]).then_inc(dve, 1)
                ve.wait_ge(dve, DVE_END + j * 4 + 3)
                ve.tensor_scalar_mul(
                    ebuf[:, :], ebuf[:, :], emx[:, 2:3]
                ).then_inc(dve, 1)

    nc.compile()
    return nc


def _prep_inputs(inputs, t_steps=T):
    bf = ml_dtypes.bfloat16
    images = np.asarray(inputs["images"], np.float32)
    captions = np.asarray(inputs["captions"])
    table = np.asarray(inputs["embed_table"], np.float32)
    W_ih = np.asarray(inputs["W_ih"], np.float32)
    W_hh = np.asarray(inputs["W_hh"], np.float32)
    W_hr = np.asarray(inputs["W_hr"], np.float32)
    bsum = (np.asarray(inputs["b_ih"], np.float32)
            + np.asarray(inputs["b_hh"], np.float32))

    P0, P1 = W_hr[0], W_hr[1]
    M00 = W_hh[0] @ P0
    M10 = W_ih[1] @ P0
    M11 = W_hh[1] @ P1

    emb = table[captions[:, :-1]]
    X = np.concatenate([images, emb], axis=1)  # [B, T, E]
    xT = np.ascontiguousarray(
        X.transpose(2, 1, 0)[:, :t_steps, :].reshape(E, t_steps * B)
    ).astype(bf)

    ones = np.zeros((128, 64), bf)
    ones[0, :] = 1
    iden = np.eye(64, dtype=np.float32).astype(bf)
    p1w = np.ascontiguousarray(P1.T.reshape(8, 128, 512)).astype(bf)

    in_maps = []
    for r in range(NCORES):
        # gate-column order per core: [i | f | o | g]  (jnp.split order i,f,g,o)
        rows = np.concatenate(
            [np.arange(g * 1024 + r * 128, g * 1024 + (r + 1) * 128)
             for g in (0, 1, 3, 2)]
        )
        w0 = np.zeros((13, 128, 512), bf)
        w0[0:4] = W_ih[0][rows].T.reshape(4, 128, 512).astype(bf)
        w0[4:12] = M00[rows].T.reshape(8, 128, 512).astype(bf)
        bt = np.zeros((128, 512), np.float32)
        bt[0, :] = bsum[0][rows]
        w0[12] = bt.astype(bf)
        w1 = np.zeros((17, 128, 512), bf)
        w1[0:8] = M10[rows].T.reshape(8, 128, 512).astype(bf)
        w1[8:16] = M11[rows].T.reshape(8, 128, 512).astype(bf)
        bt1 = np.zeros((128, 512), np.float32)
        bt1[0, :] = bsum[1][rows]
        w1[16] = bt1.astype(bf)
        in_maps.append({
            "w0": w0, "w1": w1, "p1w": p1w, "xT": xT,
            "ones": ones, "iden": iden,
            "rank": np.array([[r]], np.int32),
            "rank16": np.array([[r * (t_steps // NCORES)]], np.int32),
        })
    return in_maps


def kernel(**inputs):
    global LAST_EXEC_NS
    if TRACE:
        _install_trace_hook()
    if "nc" not in _CACHE:
        _CACHE["nc"] = build(T)
    nc = _CACHE["nc"]
    in_maps = _prep_inputs(inputs)
    res = run_bass_kernel_spmd(
        nc, in_maps, core_ids=list(range(8)), trace=TRACE
    )
    LAST_EXEC_NS = res.exec_time_ns
    out = np.concatenate([res.results[r]["y"] for r in range(8)], axis=1)
    return out.astype(np.float32)


if __name__ == "__main__":
    pass


def debug_run(inputs, t_steps=8):
    if TRACE:
        _install_trace_hook()
    nc = build(t_steps, dump=True)
    in_maps = _prep_inputs(inputs, t_steps)
    res = run_bass_kernel_spmd(nc, in_maps, core_ids=list(range(8)), trace=TRACE)
    y = np.concatenate([res.results[r]["y"] for r in range(8)], axis=1)
    s1d = [res.results[r]["s1store"] for r in range(8)]
    return y.astype(np.float32), s1d, res.exec_time_ns
